# revision 23
# baseline (speedup 1.0000x reference)
"""Trainium2 Bass kernel for nn_MLoss_68066641707785 (topk_masking loss).

Computes, for x, y of shape [128, 43264, 5] (fp32):
    m        = (y[:,:,0] > 0.5)
    face_num = sum(m)
    scale    = 1 + 1/face_num
    diff_box = scale * sum(m * (x[:,:,1:5]-y[:,:,1:5])^2) / (face_num*4)
    bce      = -(t*log(p) + (1-t)*log(1-p)),  p = x[:,:,0], t = y[:,:,0]
    diff_c   = scale * sum(m * bce) / face_num
    diff_bg  = 0.5 * mean(-log(1-p))
    out      = diff_box + diff_c + diff_bg          (scalar fp32)

Strategy: pure data-parallel over batch (16 batches/core x 8 cores).
The kernel is HBM-bound in fp32, so the host casts to bf16 (2e-2 rel-err
budget vs ~5e-5 bf16 impact), halving HBM traffic to 13.84 MB/core
(~38.7us at the 358 GB/s/core HBM limit).  bf16 also doubles DVE
tensor_tensor throughput (2x_1P mode, measured (FD/2+150)/0.96GHz).

Measured constraints that shaped this design (HW-profiled):
  - GpSimd compute steals SBUF bandwidth from DVE (concurrent DVE ops run
    ~3x slower), so GpSimd does nothing here.
  - The std tensor_tensor_reduce instr and SWDGE accum-DMAs crash the
    runtime; two DMAs into one SBUF tile also crash; STT/TS-with-accum run
    at 1x only.  So accumulating DVE ops are minimized and plain 2x TT /
    4x TS do the bulk work, with cheap accumulators on ACT.
  - ACT (scalar engine) runs 1 elem/cycle @1.2GHz for bf16: Ln/Square +
    the bg/face accumulators live there (~39us), just under DVE (~42us).
  - A single HWDGE ring (sync) saturates HBM; DMA issue stays off the
    busy ACT engine.  DMA queue order == consumption order: conf yc0
    (mask first), tiny box0 (fills the Ln-latency bubble), conf xc0,
    box1, conf g1, box2..4 (small last chunk = short tail).

Layout (host-packed, per core): conf planes in 2 groups (xc | yc planar),
box planes channel-planar in 5 uneven chunks (512,1632,1632,1224,408
cells/partition), grouped ((0,1),(2,3,4)) for the mask slices.
Per conf group g (FD = group cells):
    ACT: lp = Ln(p); lq = Ln(1-p) (accum -> bg); Identity(m) (accum -> face)
    DVE: m = (t > 0.5) [tensor_scalar, 4x]; dl = lp-lq; u = t*dl; v = u+lq
         [in-place, 2x]; w = m*v [STT, accum -> s12; bce = -(t*dl+lq)]
Per box chunk j (FD = 4*FCj):
    DVE: d = bx - yb [2x, in-place]; e = d * m_broadcast [2x, stride-0 AP]
    ACT: sq = Square(e) (accum -> se_j)
The host sums the strips in float64 and applies the final scalar formula.
Measured: 68.0us HW exec (vs 120.7us fp32 baseline; ~58us floor =
8.4us NEFF preamble + 42us DVE + tails).
"""

import numpy as np

try:
    from concourse import bacc, bass, mybir, tile
    from concourse.bass_utils import run_bass_kernel_spmd
except ImportError:  # repo not on sys.path in a fresh grading dir
    import sys

    for _p in ("/opt/trn_rl_repo", "/root/.axon_site/_ro/trn_rl_repo"):
        if _p not in sys.path:
            sys.path.insert(0, _p)
    from concourse import bacc, bass, mybir, tile
    from concourse.bass_utils import run_bass_kernel_spmd

import ml_dtypes

BF16 = ml_dtypes.bfloat16

THRESH = 0.5
ALPHA = 0.5

B, N, C = 128, 43264, 5
M = 8                      # cores
BS = B // M                # 16 batches per core
P = 128                    # SBUF partitions
CELLS = BS * N // P        # 5408 cells per partition per core
FCS = (512, 1632, 1632, 1224, 408)     # box chunk cells/partition
GROUPS = ((0, 1), (2, 3, 4))           # conf groups of box chunks
NBOX = len(FCS)
NGRP = len(GROUPS)
# acc strip columns: face[0:NGRP] s12[..] se[..] bg[..]
SE0 = 2 * NGRP
BG0 = SE0 + NBOX
ACCW = BG0 + NGRP

_CACHE = {}


def _chunk_off(j):
    return sum(FCS[:j])


def _build():
    f32 = mybir.dt.float32
    bf = mybir.dt.bfloat16
    AF = mybir.ActivationFunctionType
    OP = mybir.AluOpType

    nc = bacc.Bacc("TRN2", target_bir_lowering=False, debug=False, num_devices=M)
    cf_d = nc.declare_dram_parameter("cf", [P, 2 * CELLS], bf, isOutput=False)
    bx_d = nc.declare_dram_parameter("bx", [P, 4 * CELLS], bf, isOutput=False)
    yb_d = nc.declare_dram_parameter("yb", [P, 4 * CELLS], bf, isOutput=False)
    o_d = nc.declare_dram_parameter("o", [P, ACCW], f32, isOutput=True)
    cf_ap, bx_ap, yb_ap, o_ap = cf_d[:], bx_d[:], yb_d[:], o_d[:]

    with tile.TileContext(nc) as tc:
        with tc.tile_pool(name="io", bufs=3) as io, \
             tc.tile_pool(name="mid", bufs=2) as mid, \
             tc.tile_pool(name="acc", bufs=1) as accp:
            acc = accp.tile([P, ACCW], f32)

            # One DMA stream (sync ring; a single HWDGE ring saturates HBM
            # and keeps DMA-issue off the busy ACT engine), ordered exactly
            # by consumption: conf of group g (yc first: the mask compute
            # only needs yc), then that group's box chunks.
            ct_tiles = {}
            bx_tiles = {}

            def _conf_dmas(g, chunks):
                fg = sum(FCS[j] for j in chunks)
                cbase = 2 * _chunk_off(chunks[0])
                yct = io.tile([P, fg], bf, tag=f"yct{g}", bufs=1)
                nc.sync.dma_start(out=yct[:],
                                  in_=cf_ap[:, cbase + fg:cbase + 2 * fg])
                xct = io.tile([P, fg], bf, tag=f"xct{g}", bufs=1)
                nc.sync.dma_start(out=xct[:], in_=cf_ap[:, cbase:cbase + fg])
                ct_tiles[g] = (xct, yct)

            def _box_dmas(j):
                fc = FCS[j]
                boff = 4 * _chunk_off(j)
                bx = io.tile([P, 4 * fc], bf, tag="bx", bufs=4)
                nc.scalar.dma_start(out=bx[:], in_=bx_ap[:, boff:boff + 4 * fc])
                yb = io.tile([P, 4 * fc], bf, tag="yb", bufs=4)
                nc.sync.dma_start(out=yb[:], in_=yb_ap[:, boff:boff + 4 * fc])
                bx_tiles[j] = (bx, yb)

            # yc0 first (mask), then box0 (fills the DVE bubble while ACT
            # computes lp/lq), then xc0, then the rest in consumption order.
            fg0 = sum(FCS[j] for j in GROUPS[0])
            yct0 = io.tile([P, fg0], bf, tag="yct0", bufs=1)
            nc.sync.dma_start(out=yct0[:], in_=cf_ap[:, fg0:2 * fg0])
            _box_dmas(0)
            xct0 = io.tile([P, fg0], bf, tag="xct0", bufs=1)
            nc.sync.dma_start(out=xct0[:], in_=cf_ap[:, 0:fg0])
            ct_tiles[0] = (xct0, yct0)
            _box_dmas(1)
            _conf_dmas(1, GROUPS[1])
            for j in GROUPS[1]:
                _box_dmas(j)

            for g, chunks in enumerate(GROUPS):
                fg = sum(FCS[j] for j in chunks)     # cells in this group
                xct, yct = ct_tiles[g]
                p_ap = xct[:]
                t_ap = yct[:]

                lp = mid.tile([P, fg], bf, tag="lp")
                nc.scalar.activation(lp[:], p_ap, AF.Ln)
                lq = mid.tile([P, fg], bf, tag="lq")
                nc.scalar.activation(lq[:], p_ap, AF.Ln, bias=1.0, scale=-1.0,
                                     accum_out=acc[:, BG0 + g:BG0 + g + 1])
                m = mid.tile([P, fg], bf, tag="m")
                nc.vector.tensor_scalar(m[:], t_ap, THRESH, None, OP.is_gt)
                u = mid.tile([P, fg], bf, tag="u")
                nc.vector.tensor_sub(lp[:], lp[:], lq[:])       # lp <- dl
                nc.vector.tensor_mul(u[:], t_ap, lp[:])
                nc.vector.tensor_add(u[:], u[:], lq[:])         # u <- v
                nc.vector.scalar_tensor_tensor(
                    lp[:], m[:], 1.0, u[:], OP.mult, OP.mult,
                    accum_out=acc[:, NGRP + g:NGRP + g + 1])
                scrf = mid.tile([P, fg], bf, tag="scrf")
                nc.scalar.activation(scrf[:], m[:], AF.Identity,
                                     accum_out=acc[:, g:g + 1])
                # ---- box chunks of this group ----
                for j in chunks:
                    fc = FCS[j]
                    bx, yb = bx_tiles[j]
                    nc.vector.tensor_sub(bx[:], bx[:], yb[:])   # bx <- d
                    moff = _chunk_off(j) - _chunk_off(chunks[0])
                    m_b = m[:, moff:moff + fc].unsqueeze(1).broadcast_to(
                        (P, 4, fc))
                    nc.vector.tensor_mul(
                        bx[:].rearrange("p (c f) -> p c f", c=4),
                        bx[:].rearrange("p (c f) -> p c f", c=4), m_b)
                    nc.scalar.activation(yb[:], bx[:], AF.Square,
                                         accum_out=acc[:, SE0 + j:SE0 + j + 1])

            nc.scalar.dma_start(out=o_ap[:], in_=acc[:])

    nc.compile()
    return nc


def _get_nc():
    if "nc" not in _CACHE:
        _CACHE["nc"] = _build()
    return _CACHE["nc"]


def _in_maps(x, y):
    x = np.asarray(x, dtype=np.float32)
    y = np.asarray(y, dtype=np.float32)
    xcf = x[:, :, 0]
    ycf = y[:, :, 0]
    xbf = x[:, :, 1:5]
    ybf = y[:, :, 1:5]
    maps = []
    for i in range(M):
        sl = slice(i * BS, (i + 1) * BS)
        xc = xcf[sl].reshape(P, CELLS)
        yc = ycf[sl].reshape(P, CELLS)
        cf = np.empty((P, 2 * CELLS), dtype=BF16)
        col = 0
        for chunks in GROUPS:
            f0, f1 = _chunk_off(chunks[0]), _chunk_off(chunks[-1]) + FCS[chunks[-1]]
            fg = f1 - f0
            cf[:, col:col + fg] = xc[:, f0:f1]
            cf[:, col + fg:col + 2 * fg] = yc[:, f0:f1]
            col += 2 * fg
        # box: channel-planar within each chunk
        xb4 = xbf[sl].reshape(P, CELLS, 4)
        yb4 = ybf[sl].reshape(P, CELLS, 4)
        bx = np.empty((P, 4 * CELLS), dtype=BF16)
        yb = np.empty((P, 4 * CELLS), dtype=BF16)
        for j, fc in enumerate(FCS):
            f0 = _chunk_off(j)
            bx[:, 4 * f0:4 * (f0 + fc)] = \
                xb4[:, f0:f0 + fc, :].transpose(0, 2, 1).reshape(P, 4 * fc)
            yb[:, 4 * f0:4 * (f0 + fc)] = \
                yb4[:, f0:f0 + fc, :].transpose(0, 2, 1).reshape(P, 4 * fc)
        maps.append({"cf": cf, "bx": bx, "yb": yb})
    return maps


def _combine(outs):
    """outs: list of M arrays [P, ACCW] -> scalar fp32 loss."""
    tot = np.zeros(ACCW, dtype=np.float64)
    for o in outs:
        tot += o.astype(np.float64).sum(axis=0)
    face = tot[0:NGRP].sum()
    s12 = tot[NGRP:SE0].sum()
    se = tot[SE0:BG0].sum()
    bg = tot[BG0:ACCW].sum()
    scale = 1.0 + 1.0 / face
    diff_box = scale * se / (face * 4.0)
    diff_c = scale * (-s12) / face
    diff_bg = ALPHA * (-bg) / (B * N)
    return np.asarray(diff_box + diff_c + diff_bg, dtype=np.float32)


def kernel(x, y, **run_kwargs):
    nc = _get_nc()
    res = run_bass_kernel_spmd(nc, _in_maps(x, y), core_ids=list(range(M)),
                               **run_kwargs)
    out = _combine([res.results[i]["o"] for i in range(M)])
    if run_kwargs:
        return out, res
    return out
